# revision 9
# baseline (speedup 1.0000x reference)
"""BiMamba block Trainium2 kernel.

Contract: kernel(**inputs) takes FULL inputs (shapes from the problem spec),
returns the FULL (8, 2048, 128) float32 output. Internally shards
data-parallel over batch across 8 NeuronCores and runs a Bass/Tile kernel.

Layout per core (batch element b):
  - everything lives as [128 partitions, L free] tiles; d_inner=256 is split
    into two halves of 128 channels.
  - LayerNorm via PE column-sum matmuls (mean/var broadcast through PSUM).
  - causal depthwise conv4 folded into the in_proj matmul: 4 shifted
    PSUM-accumulated matmuls with weights conv_w[d,k] * in_proj_w[d,:].
  - softplus(x) = Ln(Exp(x) + 1) on ACT (same table set as the scan's exps).
  - selective scan: per (half, state) group g the recurrence
    h[l] = exp(dt*A_g)*h[l-1] + (dt*u)*B runs as one hardware
    tensor_tensor_scan along the free dim (fp32 internal state, bf16 data).
  - y = sum_s C_s * h_s accumulated in PSUM via identity-matmul on PE;
    u*Dp folded in as one extra accumulated matmul with lhsT=diag(Dp).
  - backward direction: same code with negative-stride (reversed) APs into
    the scan; conv uses right-shifted taps. No explicit flips.
"""
import sys
import time

sys.path.insert(0, "/opt/trn_rl_repo")

import numpy as np

B_SZ, L, D_MODEL = 8, 2048, 128
D_STATE, D_CONV = 48, 4
D_INNER = 256
DT_RANK = 8
N_CORES = 8
EPS = 1e-5
NCH = 4  # 512-column chunks per L

X_ON_GP = True    # X = dtu*Bb multiply on GPSIMD (else DVE)

_CACHE = {}


# ---------------------------------------------------------------- device ---
def _build_nc():
    import concourse.bacc as bacc
    import concourse.tile as tile
    from concourse import mybir
    from contextlib import ExitStack

    F32 = mybir.dt.float32
    BF = mybir.dt.bfloat16
    AF = mybir.ActivationFunctionType
    OP = mybir.AluOpType

    nc = bacc.Bacc("TRN2", target_bir_lowering=False, debug=False)

    def din(name, shape, dt=F32):
        return nc.dram_tensor(name, shape, dt, kind="ExternalInput")

    d_x = din("x_T", [128, L])
    d_wci = {"f": din("wci_f", [128, 1024]), "b": din("wci_b", [128, 1024])}
    d_wz = din("wz", [128, 256])
    d_wx = {"f": din("wx_f", [128, 208], BF), "b": din("wx_b", [128, 208], BF)}
    d_wdt = {"f": din("wdt_f", [8, 256], BF), "b": din("wdt_b", [8, 256], BF)}
    d_wout = din("wout", [128, 256], BF)
    d_A = {"f": din("A_f", [128, 96]), "b": din("A_b", [128, 96])}
    # cols: ln_w, ln_b, dtb_f(2), dtb_b(2), cb_f(2), cb_b(2), eps, one
    d_cols = din("cols", [128, 12])
    d_ones = din("ones128", [128, 128])      # all 1/128
    d_ones1 = din("ones_1x128", [1, 128])    # all 1.0
    d_ident = din("identv", [128, 128], BF)  # identity
    d_dpd = {"f": din("dpd_f", [128, 256], BF),  # diag(Dp) halves
             "b": din("dpd_b", [128, 256], BF)}
    d_out = nc.dram_tensor("out_T", [128, L], F32, kind="ExternalOutput")

    C512 = [slice(c * 512, (c + 1) * 512) for c in range(NCH)]

    with tile.TileContext(nc) as tc:
        with ExitStack() as ctx:
            consts = ctx.enter_context(tc.tile_pool(name="consts", bufs=1))
            pers = ctx.enter_context(tc.tile_pool(name="pers", bufs=1))
            bigf = ctx.enter_context(tc.tile_pool(name="bigf", bufs=2))
            v512 = ctx.enter_context(tc.tile_pool(name="v512", bufs=4))
            stg = ctx.enter_context(tc.tile_pool(name="stg", bufs=2))
            pAM = ctx.enter_context(tc.tile_pool(name="pAM", bufs=3))
            pX = ctx.enter_context(tc.tile_pool(name="pX", bufs=2))
            pH = ctx.enter_context(tc.tile_pool(name="pH", bufs=2))
            pB = ctx.enter_context(tc.tile_pool(name="pB", bufs=2))
            pC = ctx.enter_context(tc.tile_pool(name="pC", bufs=2))
            ps = ctx.enter_context(tc.tile_pool(name="ps", bufs=2, space="PSUM"))

            def cload(d, shape, tag, dt=F32):
                t = consts.tile(shape, dt, tag=tag, name=f"c_{tag}")
                nc.gpsimd.dma_start(out=t[:, :], in_=d[:, :])
                return t

            ones_sb = cload(d_ones, [128, 128], "ones")
            ones1_sb = cload(d_ones1, [1, 128], "ones1")
            ident_sb = cload(d_ident, [128, 128], "ident", BF)
            wci = {k: cload(d_wci[k], [128, 1024], f"wci_{k}") for k in "fb"}
            wz_sb = cload(d_wz, [128, 256], "wz")
            wx = {k: cload(d_wx[k], [128, 208], f"wx_{k}", BF) for k in "fb"}
            wdt = {k: cload(d_wdt[k], [8, 256], f"wdt_{k}", BF) for k in "fb"}
            wout_sb = cload(d_wout, [128, 256], "wout", BF)
            A_sb = {k: cload(d_A[k], [128, 96], f"A_{k}") for k in "fb"}
            dpd = {k: cload(d_dpd[k], [128, 256], f"dpd_{k}", BF) for k in "fb"}
            cols = cload(d_cols, [128, 12], "cols")
            lnw, lnb = cols[:, 0:1], cols[:, 1:2]
            dtb = {"f": (cols[:, 2:3], cols[:, 3:4]),
                   "b": (cols[:, 4:5], cols[:, 5:6])}
            cb = {"f": (cols[:, 6:7], cols[:, 7:8]),
                  "b": (cols[:, 8:9], cols[:, 9:10])}
            c_eps, c_one = cols[:, 10:11], cols[:, 11:12]

            # ---------------- LayerNorm (D in partitions, stats via PE) ----
            t_x = pers.tile([128, L], F32, tag="x")
            nc.gpsimd.dma_start(out=t_x[:, :], in_=d_x[:, :])

            p_mu = ps.tile([128, L], F32, tag="ps", name="p_mu")
            for c in range(NCH):
                nc.tensor.matmul(p_mu[:, C512[c]], ones_sb[:, :],
                                 t_x[:, C512[c]], start=True, stop=True)
            t_xc = bigf.tile([128, L], F32, tag="bigf", name="xcen")
            nc.vector.tensor_tensor(t_xc[:, :], t_x[:, :], p_mu[:, :],
                                    op=OP.subtract)
            t_sq = bigf.tile([128, L], F32, tag="bigf", name="sq")
            nc.scalar.square(t_sq[:, :], t_xc[:, :])
            p_var = ps.tile([128, L], F32, tag="ps", name="p_var")
            for c in range(NCH):
                nc.tensor.matmul(p_var[0:1, C512[c]], ones_sb[:, 0:1],
                                 t_sq[:, C512[c]], start=True, stop=True)
            p_rb = ps.tile([128, L], F32, tag="ps", name="p_rb")
            for c in range(NCH):
                lnr = v512.tile([1, 512], F32, tag="v512", name=f"lnr{c}")
                nc.scalar.activation(lnr[0:1, :], p_var[0:1, C512[c]], AF.Ln,
                                     bias=c_eps[0:1, :])
                rst = v512.tile([1, 512], F32, tag="v512", name=f"rst{c}")
                nc.scalar.activation(rst[0:1, :], lnr[0:1, :], AF.Exp,
                                     scale=-0.5)
                nc.tensor.matmul(p_rb[:, C512[c]], ones1_sb[:, :],
                                 rst[0:1, :], start=True, stop=True)
            t_xnp = pers.tile([128, L + 6], F32, tag="xnpad")
            nc.vector.memset(t_xnp[:, 0:3], 0.0)
            nc.vector.memset(t_xnp[:, L + 3:L + 6], 0.0)
            t_xn0 = bigf.tile([128, L], F32, tag="bigf", name="xn0")
            nc.vector.tensor_tensor(t_xn0[:, :], t_xc[:, :], p_rb[:, :],
                                    op=OP.mult)
            nc.scalar.activation(t_xnp[:, 3:L + 3], t_xn0[:, :], AF.Identity,
                                 bias=lnb, scale=lnw)

            # ---------------- projections --------------------------------
            # conv-folded in_proj -> silu -> u (bf16), chunked through PSUM.
            # fwd taps read xn shifted by k-3 (left pad), bwd by 3-k (right).
            t_u = {}
            for dk in "fb":
                for h in range(2):
                    u = pers.tile([128, L], BF, tag=f"u_{dk}{h}",
                                  name=f"u_{dk}{h}")
                    t_u[(dk, h)] = u
                    for c in range(NCH):
                        pu = ps.tile([128, 512], F32, tag="ps",
                                     name=f"pu{dk}{h}{c}")
                        for k in range(4):
                            off = (c * 512 + k) if dk == "f" else (c * 512 + 6 - k)
                            nc.tensor.matmul(
                                pu[:, :],
                                wci[dk][:, (k * 2 + h) * 128:(k * 2 + h + 1) * 128],
                                t_xnp[:, off:off + 512],
                                start=(k == 0), stop=(k == 3))
                        uc = v512.tile([128, 512], F32, tag="v512",
                                       name=f"uc{dk}{h}{c}")
                        nc.scalar.activation(uc[:, :], pu[:, :],
                                             AF.Identity, bias=cb[dk][h])
                        sg = v512.tile([128, 512], F32, tag="v512",
                                       name=f"sg{dk}{h}{c}")
                        nc.scalar.activation(sg[:, :], uc[:, :], AF.Sigmoid)
                        nc.vector.tensor_tensor(u[:, C512[c]], uc[:, :],
                                                sg[:, :], op=OP.mult)
            # z projection -> silu -> sz (bf16), shared by both dirs
            t_sz = {}
            for h in range(2):
                sz = pers.tile([128, L], BF, tag=f"sz{h}", name=f"sz{h}")
                t_sz[h] = sz
                for c in range(NCH):
                    pz = ps.tile([128, 512], F32, tag="ps", name=f"pz{h}{c}")
                    nc.tensor.matmul(pz[:, :], wz_sb[:, h * 128:(h + 1) * 128],
                                     t_xnp[:, 3 + c * 512:3 + (c + 1) * 512],
                                     start=True, stop=True)
                    zc = v512.tile([128, 512], F32, tag="v512", name=f"zc{h}{c}")
                    nc.scalar.copy(zc[:, :], pz[:, :])
                    sg = v512.tile([128, 512], F32, tag="v512", name=f"sgz{h}{c}")
                    nc.scalar.activation(sg[:, :], zc[:, :], AF.Sigmoid)
                    nc.vector.tensor_tensor(sz[:, C512[c]], zc[:, :],
                                            sg[:, :], op=OP.mult)

            # x_proj -> xd (bf16) [104, L]: rows 0:8 dt_raw, 8:56 B, 56:104 C
            t_xd = {}
            for dk in "fb":
                xd = pers.tile([104, L], BF, tag=f"xd_{dk}", name=f"xd_{dk}")
                for c in range(NCH):
                    px = ps.tile([128, 512], F32, tag="ps", name=f"px{dk}{c}")
                    for h in range(2):
                        nc.tensor.matmul(px[0:104, :],
                                         wx[dk][:, h * 104:(h + 1) * 104],
                                         t_u[(dk, h)][:, C512[c]],
                                         start=(h == 0), stop=(h == 1))
                    nc.scalar.copy(xd[:, C512[c]], px[0:104, :])
                t_xd[dk] = xd

            # dt = softplus(dt_raw @ wdt.T + dtb) = Ln(Exp(.) + 1), bf16
            t_dt = {}
            for dk in "fb":
                for h in range(2):
                    dt_t = pers.tile([128, L], BF, tag=f"dt_{dk}{h}",
                                     name=f"dt_{dk}{h}")
                    for c in range(NCH):
                        pd = ps.tile([128, 512], F32, tag="ps",
                                     name=f"pd{dk}{h}{c}")
                        nc.tensor.matmul(pd[:, :],
                                         wdt[dk][:, h * 128:(h + 1) * 128],
                                         t_xd[dk][0:8, C512[c]],
                                         start=True, stop=True)
                        te = v512.tile([128, 512], F32, tag="v512",
                                       name=f"te{dk}{h}{c}")
                        nc.scalar.activation(te[:, :], pd[:, :], AF.Exp,
                                             bias=dtb[dk][h])
                        nc.scalar.activation(dt_t[:, C512[c]], te[:, :],
                                             AF.Ln, bias=c_one)
                    t_dt[(dk, h)] = dt_t
            # dtu = dt * u (bf16)
            t_dtu = {}
            for dk in "fb":
                for h in range(2):
                    dtu = pers.tile([128, L], BF, tag=f"dtu_{dk}{h}",
                                    name=f"dtu_{dk}{h}")
                    nc.vector.tensor_tensor(dtu[:, :], t_dt[(dk, h)][:, :],
                                            t_u[(dk, h)][:, :], op=OP.mult)
                    t_dtu[(dk, h)] = dtu

            # ---------------- scan loops ---------------------------------
            def rev(ap):
                return ap[:, ::-1]

            t_yf = {}
            t_yg = {}
            for dk in "fb":
                p_y = {}
                for h in range(2):
                    p_y[h] = ps.tile([128, L], F32, tag="ps",
                                     name=f"py_{dk}{h}")
                xd = t_xd[dk]
                for s in range(D_STATE):
                    sB = stg.tile([1, L], BF, tag="stg", name=f"sB{dk}{s}")
                    nc.sync.dma_start(out=sB[:, :], in_=xd[8 + s:9 + s, :])
                    Bb = pB.tile([128, L], BF, tag="Bb", name=f"Bb{dk}{s}")
                    nc.gpsimd.partition_broadcast(Bb[:, :], sB[0:1, :])
                    sC = stg.tile([1, L], BF, tag="stg", name=f"sC{dk}{s}")
                    nc.sync.dma_start(out=sC[:, :], in_=xd[56 + s:57 + s, :])
                    Cb = pC.tile([128, L], BF, tag="Cb", name=f"Cb{dk}{s}")
                    nc.gpsimd.partition_broadcast(Cb[:, :], sC[0:1, :])
                    for h in range(2):
                        g = h * 48 + s
                        dA = pAM.tile([128, L], BF, tag="AM",
                                      name=f"dA{dk}{s}{h}")
                        nc.scalar.activation(dA[:, :], t_dt[(dk, h)][:, :],
                                             AF.Exp, scale=A_sb[dk][:, g:g + 1])
                        Xt = pX.tile([128, L], BF, tag="X", name=f"X{dk}{s}{h}")
                        eng = nc.gpsimd if X_ON_GP else nc.vector
                        eng.tensor_tensor(Xt[:, :], t_dtu[(dk, h)][:, :],
                                          Bb[:, :], op=OP.mult)
                        Ht = pH.tile([128, L], BF, tag="H", name=f"H{dk}{s}{h}")
                        if dk == "f":
                            nc.vector.tensor_tensor_scan(
                                Ht[:, :], dA[:, :], Xt[:, :], 0.0,
                                OP.mult, OP.add)
                        else:
                            nc.vector.tensor_tensor_scan(
                                rev(Ht), rev(dA), rev(Xt), 0.0,
                                OP.mult, OP.add)
                        Mt = pAM.tile([128, L], BF, tag="AM",
                                      name=f"M{dk}{s}{h}")
                        nc.vector.tensor_tensor(Mt[:, :], Ht[:, :], Cb[:, :],
                                                op=OP.mult)
                        for c in range(NCH):
                            nc.tensor.matmul(p_y[h][:, C512[c]], ident_sb[:, :],
                                             Mt[:, C512[c]],
                                             start=(s == 0), stop=False)
                # fold u*Dp in as the final accumulated matmul (lhsT=diag(Dp))
                for h in range(2):
                    for c in range(NCH):
                        nc.tensor.matmul(p_y[h][:, C512[c]],
                                         dpd[dk][:, h * 128:(h + 1) * 128],
                                         t_u[(dk, h)][:, C512[c]],
                                         start=False, stop=True)
                if dk == "f":
                    for h in range(2):
                        yf = pers.tile([128, L], F32, tag=f"yf{h}",
                                       name=f"yf{h}")
                        nc.vector.tensor_copy(yf[:, :], p_y[h][:, :])
                        t_yf[h] = yf
                else:
                    # combine: yg = (yf + yb) * sz   (bf16)
                    for h in range(2):
                        yfb = pX.tile([128, L], BF, tag="X", name=f"yfb{h}")
                        nc.vector.tensor_tensor(yfb[:, :], t_yf[h][:, :],
                                                p_y[h][:, :], op=OP.add)
                        yg = pH.tile([128, L], BF, tag="H", name=f"yg{h}")
                        nc.vector.tensor_tensor(yg[:, :], yfb[:, :],
                                                t_sz[h][:, :], op=OP.mult)
                        t_yg[h] = yg

            # ---------------- out proj + residual -------------------------
            t_out = bigf.tile([128, L], F32, tag="bigf", name="outsb")
            for c in range(NCH):
                po = ps.tile([128, 512], F32, tag="ps", name=f"po{c}")
                for h in range(2):
                    nc.tensor.matmul(po[:, :], wout_sb[:, h * 128:(h + 1) * 128],
                                     t_yg[h][:, C512[c]],
                                     start=(h == 0), stop=(h == 1))
                nc.vector.tensor_tensor(t_out[:, C512[c]], po[:, :],
                                        t_x[:, C512[c]], op=OP.add)
            nc.gpsimd.dma_start(out=d_out[:, :], in_=t_out[:, :])

    nc.compile()
    return nc


# ---------------------------------------------------------------- host -----
def _prep_weights(inputs):
    import ml_dtypes
    f = np.float32
    bf = ml_dtypes.bfloat16
    ip = inputs["in_proj_w"].astype(f)          # (512, 128)
    xc_w = ip[:D_INNER]                          # (256, 128)
    z_w = ip[D_INNER:]                           # (256, 128)

    def conv_fold(conv_w):
        # wci[p, (k*2+h)*128 + m] = conv_w[h*128+m, 0, k] * xc_w[h*128+m, p]
        out = np.zeros((128, 1024), f)
        for k in range(4):
            for h in range(2):
                blk = (conv_w[h * 128:(h + 1) * 128, 0, k][:, None]
                       * xc_w[h * 128:(h + 1) * 128, :])   # (128m, 128p)
                out[:, (k * 2 + h) * 128:(k * 2 + h + 1) * 128] = blk.T
        return out

    def xproj_pack(w):                           # w: (104, 256)
        out = np.zeros((128, 208), f)
        for h in range(2):
            out[:, h * 104:(h + 1) * 104] = w[:, h * 128:(h + 1) * 128].T
        return out.astype(bf)

    wz = np.zeros((128, 256), f)
    for h in range(2):
        wz[:, h * 128:(h + 1) * 128] = z_w[h * 128:(h + 1) * 128, :].T
    wout = np.zeros((128, 256), f)
    op = inputs["out_proj_w"].astype(f)          # (128, 256)
    for h in range(2):
        wout[:, h * 128:(h + 1) * 128] = op[:, h * 128:(h + 1) * 128].T

    def A_pack(A_log):
        A = -np.exp(A_log.astype(f))             # (256, 48)
        out = np.zeros((128, 96), f)
        for h in range(2):
            out[:, h * 48:(h + 1) * 48] = A[h * 128:(h + 1) * 128, :]
        return out

    def halves(v):                               # (256,) -> (128, 2)
        return np.stack([v[:128], v[128:]], axis=1).astype(f)

    def dp_diag(v):                              # (256,) -> [128, 256] bf16
        out = np.zeros((128, 256), f)
        out[:, 0:128] = np.diag(v[:128])
        out[:, 128:256] = np.diag(v[128:])
        return out.astype(bf)

    cols = np.zeros((128, 12), f)
    cols[:, 0] = inputs["ln_w"]
    cols[:, 1] = inputs["ln_b"]
    cols[:, 2:4] = halves(inputs["dt_proj_b"])
    cols[:, 4:6] = halves(inputs["dt_proj_b_b"])
    cols[:, 6:8] = halves(inputs["conv_b"])
    cols[:, 8:10] = halves(inputs["conv_b_b"])
    cols[:, 10] = EPS
    cols[:, 11] = 1.0

    w = {
        "wci_f": conv_fold(inputs["conv_w"].astype(f)),
        "wci_b": conv_fold(inputs["conv_w_b"].astype(f)),
        "wz": wz,
        "wx_f": xproj_pack(inputs["x_proj_w"].astype(f)),
        "wx_b": xproj_pack(inputs["x_proj_w_b"].astype(f)),
        "wdt_f": inputs["dt_proj_w"].astype(f).T.copy().astype(bf),  # (8,256)
        "wdt_b": inputs["dt_proj_w_b"].astype(f).T.copy().astype(bf),
        "wout": wout.astype(bf),
        "A_f": A_pack(inputs["A_log"]),
        "A_b": A_pack(inputs["A_b_log"]),
        "cols": cols,
        "ones128": np.full((128, 128), 1.0 / 128.0, f),
        "ones_1x128": np.ones((1, 128), f),
        "identv": np.eye(128).astype(bf),
        "dpd_f": dp_diag(inputs["Dp"].astype(f)),
        "dpd_b": dp_diag(inputs["Dp_b"].astype(f)),
    }
    return w


def _get_nc():
    if "nc" not in _CACHE:
        _CACHE["nc"] = _build_nc()
    return _CACHE["nc"]


def _run(in_maps, n_iter=1):
    """Execute on 8 cores; returns (results, best_wall_ns)."""
    from concourse import bass2jax
    nc = _get_nc()
    best = None
    results = None
    for _ in range(n_iter):
        t0 = time.perf_counter_ns()
        results = bass2jax.run_bass_via_pjrt(nc, in_maps, n_cores=N_CORES)
        dt_ns = time.perf_counter_ns() - t0
        best = dt_ns if best is None else min(best, dt_ns)
    return results, best


def kernel(**inputs):
    inputs = {k: np.asarray(v) for k, v in inputs.items()}
    w = _prep_weights(inputs)
    x = inputs["x"].astype(np.float32)           # (8, 2048, 128)
    in_maps = []
    for b in range(N_CORES):
        m = dict(w)
        m["x_T"] = np.ascontiguousarray(x[b].T)  # (128, 2048)
        in_maps.append(m)
    results, wall = _run(in_maps, n_iter=1)
    _CACHE["last_wall_ns"] = wall
    out = np.stack([results[b]["out_T"].T for b in range(N_CORES)])
    return out.astype(np.float32)


# revision 11
# speedup vs baseline: 1.4755x; 1.4755x over previous
"""BiMamba block Trainium2 kernel.

Contract: kernel(**inputs) takes FULL inputs (shapes from the problem spec),
returns the FULL (8, 2048, 128) float32 output. Internally shards
data-parallel over batch across 8 NeuronCores and runs a Bass/Tile kernel.

Layout per core (batch element b):
  - everything lives as [128 partitions, L free] tiles; d_inner=256 is split
    into two halves of 128 channels.
  - LayerNorm via PE column-sum matmuls (mean/var broadcast through PSUM).
  - causal depthwise conv4 folded into the in_proj matmul: 4 shifted
    PSUM-accumulated matmuls with weights conv_w[d,k] * in_proj_w[d,:].
  - softplus(x) = Ln(Exp(x) + 1) on ACT (same table set as the scan's exps).
  - selective scan: per (half, state) group g the recurrence
    h[l] = exp(dt*A_g)*h[l-1] + (dt*u)*B runs as one hardware
    tensor_tensor_scan along the free dim (fp32 internal state, bf16 data).
  - y = sum_s C_s * h_s accumulated in PSUM via identity-matmul on PE;
    u*Dp folded in as one extra accumulated matmul with lhsT=diag(Dp).
  - backward direction: same code with negative-stride (reversed) APs into
    the scan; conv uses right-shifted taps. No explicit flips.
"""
import sys
import time

sys.path.insert(0, "/opt/trn_rl_repo")

import numpy as np

B_SZ, L, D_MODEL = 8, 2048, 128
D_STATE, D_CONV = 48, 4
D_INNER = 256
DT_RANK = 8
N_CORES = 8
EPS = 1e-5
NCH = 4  # 512-column chunks per L

X_ON_GP = True    # X = dtu*Bb multiply on GPSIMD (else DVE)

_CACHE = {}


# ---------------------------------------------------------------- device ---
def _build_nc():
    import concourse.bacc as bacc
    import concourse.tile as tile
    from concourse import mybir
    from contextlib import ExitStack

    F32 = mybir.dt.float32
    BF = mybir.dt.bfloat16
    AF = mybir.ActivationFunctionType
    OP = mybir.AluOpType

    nc = bacc.Bacc("TRN2", target_bir_lowering=False, debug=False)

    def din(name, shape, dt=F32):
        return nc.dram_tensor(name, shape, dt, kind="ExternalInput")

    d_x = din("x_T", [128, L])
    d_wci = {"f": din("wci_f", [128, 1024]), "b": din("wci_b", [128, 1024])}
    d_wz = din("wz", [128, 256])
    d_wx = {"f": din("wx_f", [128, 208], BF), "b": din("wx_b", [128, 208], BF)}
    d_wdt = {"f": din("wdt_f", [8, 256], BF), "b": din("wdt_b", [8, 256], BF)}
    d_wout = din("wout", [128, 256], BF)
    d_A = {"f": din("A_f", [128, 96]), "b": din("A_b", [128, 96])}
    # cols: ln_w, ln_b, dtb_f(2), dtb_b(2), cb_f(2), cb_b(2), eps, one
    d_cols = din("cols", [128, 12])
    d_ones = din("ones128", [128, 128])      # all 1/128
    d_ones1 = din("ones_1x128", [1, 128])    # all 1.0
    d_ident = din("identv", [128, 128], BF)  # identity
    d_dpd = {"f": din("dpd_f", [128, 256], BF),  # diag(Dp) halves
             "b": din("dpd_b", [128, 256], BF)}
    d_out = nc.dram_tensor("out_T", [128, L], F32, kind="ExternalOutput")

    C512 = [slice(c * 512, (c + 1) * 512) for c in range(NCH)]

    with tile.TileContext(nc) as tc:
        with ExitStack() as ctx:
            consts = ctx.enter_context(tc.tile_pool(name="consts", bufs=1))
            pers = ctx.enter_context(tc.tile_pool(name="pers", bufs=1))
            bigf = ctx.enter_context(tc.tile_pool(name="bigf", bufs=2))
            v512 = ctx.enter_context(tc.tile_pool(name="v512", bufs=4))
            stg = ctx.enter_context(tc.tile_pool(name="stg", bufs=2))
            pAM = ctx.enter_context(tc.tile_pool(name="pAM", bufs=3))
            pX = ctx.enter_context(tc.tile_pool(name="pX", bufs=2))
            pH = ctx.enter_context(tc.tile_pool(name="pH", bufs=2))
            pB = ctx.enter_context(tc.tile_pool(name="pB", bufs=2))
            pC = ctx.enter_context(tc.tile_pool(name="pC", bufs=2))
            ps = ctx.enter_context(tc.tile_pool(name="ps", bufs=2, space="PSUM"))

            def cload(d, shape, tag, dt=F32):
                t = consts.tile(shape, dt, tag=tag, name=f"c_{tag}")
                nc.gpsimd.dma_start(out=t[:, :], in_=d[:, :])
                return t

            ones_sb = cload(d_ones, [128, 128], "ones")
            ones1_sb = cload(d_ones1, [1, 128], "ones1")
            ident_sb = cload(d_ident, [128, 128], "ident", BF)
            wci = {k: cload(d_wci[k], [128, 1024], f"wci_{k}") for k in "fb"}
            wz_sb = cload(d_wz, [128, 256], "wz")
            wx = {k: cload(d_wx[k], [128, 208], f"wx_{k}", BF) for k in "fb"}
            wdt = {k: cload(d_wdt[k], [8, 256], f"wdt_{k}", BF) for k in "fb"}
            wout_sb = cload(d_wout, [128, 256], "wout", BF)
            A_sb = {k: cload(d_A[k], [128, 96], f"A_{k}") for k in "fb"}
            dpd = {k: cload(d_dpd[k], [128, 256], f"dpd_{k}", BF) for k in "fb"}
            cols = cload(d_cols, [128, 12], "cols")
            lnw, lnb = cols[:, 0:1], cols[:, 1:2]
            dtb = {"f": (cols[:, 2:3], cols[:, 3:4]),
                   "b": (cols[:, 4:5], cols[:, 5:6])}
            cb = {"f": (cols[:, 6:7], cols[:, 7:8]),
                  "b": (cols[:, 8:9], cols[:, 9:10])}
            c_eps, c_one = cols[:, 10:11], cols[:, 11:12]

            # ---------------- LayerNorm (D in partitions, stats via PE) ----
            t_x = pers.tile([128, L], F32, tag="x")
            nc.gpsimd.dma_start(out=t_x[:, :], in_=d_x[:, :])

            p_mu = ps.tile([128, L], F32, tag="ps", name="p_mu")
            for c in range(NCH):
                nc.tensor.matmul(p_mu[:, C512[c]], ones_sb[:, :],
                                 t_x[:, C512[c]], start=True, stop=True)
            t_xc = bigf.tile([128, L], F32, tag="bigf", name="xcen")
            nc.vector.tensor_tensor(t_xc[:, :], t_x[:, :], p_mu[:, :],
                                    op=OP.subtract)
            t_sq = bigf.tile([128, L], F32, tag="bigf", name="sq")
            nc.scalar.square(t_sq[:, :], t_xc[:, :])
            p_var = ps.tile([128, L], F32, tag="ps", name="p_var")
            for c in range(NCH):
                nc.tensor.matmul(p_var[0:1, C512[c]], ones_sb[:, 0:1],
                                 t_sq[:, C512[c]], start=True, stop=True)
            p_rb = ps.tile([128, L], F32, tag="ps", name="p_rb")
            for c in range(NCH):
                lnr = v512.tile([1, 512], F32, tag="v512", name=f"lnr{c}")
                nc.scalar.activation(lnr[0:1, :], p_var[0:1, C512[c]], AF.Ln,
                                     bias=c_eps[0:1, :])
                rst = v512.tile([1, 512], F32, tag="v512", name=f"rst{c}")
                nc.scalar.activation(rst[0:1, :], lnr[0:1, :], AF.Exp,
                                     scale=-0.5)
                nc.tensor.matmul(p_rb[:, C512[c]], ones1_sb[:, :],
                                 rst[0:1, :], start=True, stop=True)
            t_xnp = pers.tile([128, L + 6], F32, tag="xnpad")
            nc.vector.memset(t_xnp[:, 0:3], 0.0)
            nc.vector.memset(t_xnp[:, L + 3:L + 6], 0.0)
            t_xn0 = bigf.tile([128, L], F32, tag="bigf", name="xn0")
            nc.vector.tensor_tensor(t_xn0[:, :], t_xc[:, :], p_rb[:, :],
                                    op=OP.mult)
            nc.scalar.activation(t_xnp[:, 3:L + 3], t_xn0[:, :], AF.Identity,
                                 bias=lnb, scale=lnw)

            # ---------------- projections --------------------------------
            # conv-folded in_proj -> silu -> u (bf16), chunked through PSUM.
            # fwd taps read xn shifted by k-3 (left pad), bwd by 3-k (right).
            t_u = {}
            for dk in "fb":
                for h in range(2):
                    u = pers.tile([128, L], BF, tag=f"u_{dk}{h}",
                                  name=f"u_{dk}{h}")
                    t_u[(dk, h)] = u
                    for c in range(NCH):
                        pu = ps.tile([128, 512], F32, tag="ps",
                                     name=f"pu{dk}{h}{c}")
                        for k in range(4):
                            off = (c * 512 + k) if dk == "f" else (c * 512 + 6 - k)
                            nc.tensor.matmul(
                                pu[:, :],
                                wci[dk][:, (k * 2 + h) * 128:(k * 2 + h + 1) * 128],
                                t_xnp[:, off:off + 512],
                                start=(k == 0), stop=(k == 3))
                        uc = v512.tile([128, 512], F32, tag="v512",
                                       name=f"uc{dk}{h}{c}")
                        nc.scalar.activation(uc[:, :], pu[:, :],
                                             AF.Identity, bias=cb[dk][h])
                        sg = v512.tile([128, 512], F32, tag="v512",
                                       name=f"sg{dk}{h}{c}")
                        nc.scalar.activation(sg[:, :], uc[:, :], AF.Sigmoid)
                        nc.vector.tensor_tensor(u[:, C512[c]], uc[:, :],
                                                sg[:, :], op=OP.mult)
            # z projection -> silu -> sz (bf16), shared by both dirs
            t_sz = {}
            for h in range(2):
                sz = pers.tile([128, L], BF, tag=f"sz{h}", name=f"sz{h}")
                t_sz[h] = sz
                for c in range(NCH):
                    pz = ps.tile([128, 512], F32, tag="ps", name=f"pz{h}{c}")
                    nc.tensor.matmul(pz[:, :], wz_sb[:, h * 128:(h + 1) * 128],
                                     t_xnp[:, 3 + c * 512:3 + (c + 1) * 512],
                                     start=True, stop=True)
                    zc = v512.tile([128, 512], F32, tag="v512", name=f"zc{h}{c}")
                    nc.scalar.copy(zc[:, :], pz[:, :])
                    sg = v512.tile([128, 512], F32, tag="v512", name=f"sgz{h}{c}")
                    nc.scalar.activation(sg[:, :], zc[:, :], AF.Sigmoid)
                    nc.vector.tensor_tensor(sz[:, C512[c]], zc[:, :],
                                            sg[:, :], op=OP.mult)

            # x_proj -> xd (bf16) [104, L]: rows 0:8 dt_raw, 8:56 B, 56:104 C
            t_xd = {}
            for dk in "fb":
                xd = pers.tile([104, L], BF, tag=f"xd_{dk}", name=f"xd_{dk}")
                for c in range(NCH):
                    px = ps.tile([128, 512], F32, tag="ps", name=f"px{dk}{c}")
                    for h in range(2):
                        nc.tensor.matmul(px[0:104, :],
                                         wx[dk][:, h * 104:(h + 1) * 104],
                                         t_u[(dk, h)][:, C512[c]],
                                         start=(h == 0), stop=(h == 1))
                    nc.scalar.copy(xd[:, C512[c]], px[0:104, :])
                t_xd[dk] = xd

            # dt = softplus(dt_raw @ wdt.T + dtb) = Ln(Exp(.) + 1), bf16
            t_dt = {}
            for dk in "fb":
                for h in range(2):
                    dt_t = pers.tile([128, L], BF, tag=f"dt_{dk}{h}",
                                     name=f"dt_{dk}{h}")
                    for c in range(NCH):
                        pd = ps.tile([128, 512], F32, tag="ps",
                                     name=f"pd{dk}{h}{c}")
                        nc.tensor.matmul(pd[:, :],
                                         wdt[dk][:, h * 128:(h + 1) * 128],
                                         t_xd[dk][0:8, C512[c]],
                                         start=True, stop=True)
                        te = v512.tile([128, 512], F32, tag="v512",
                                       name=f"te{dk}{h}{c}")
                        nc.scalar.activation(te[:, :], pd[:, :], AF.Exp,
                                             bias=dtb[dk][h])
                        nc.scalar.activation(dt_t[:, C512[c]], te[:, :],
                                             AF.Ln, bias=c_one)
                    t_dt[(dk, h)] = dt_t
            # dtu = dt * u (bf16)
            t_dtu = {}
            for dk in "fb":
                for h in range(2):
                    dtu = pers.tile([128, L], BF, tag=f"dtu_{dk}{h}",
                                    name=f"dtu_{dk}{h}")
                    nc.vector.tensor_tensor(dtu[:, :], t_dt[(dk, h)][:, :],
                                            t_u[(dk, h)][:, :], op=OP.mult)
                    t_dtu[(dk, h)] = dtu

            # ---------------- scan loops ---------------------------------
            def rev(ap):
                return ap[:, ::-1]

            t_yf = {}
            t_yg = {}
            for dk in "fb":
                p_y = {}
                for h in range(2):
                    p_y[h] = ps.tile([128, L], F32, tag="ps",
                                     name=f"py_{dk}{h}")
                xd = t_xd[dk]
                for s in range(D_STATE):
                    sB = stg.tile([1, L], BF, tag="stg", name=f"sB{dk}{s}")
                    nc.sync.dma_start(out=sB[:, :], in_=xd[8 + s:9 + s, :])
                    Bb = pB.tile([128, L], BF, tag="Bb", name=f"Bb{dk}{s}")
                    nc.gpsimd.partition_broadcast(Bb[:, :], sB[0:1, :])
                    sC = stg.tile([1, L], BF, tag="stg", name=f"sC{dk}{s}")
                    nc.sync.dma_start(out=sC[:, :], in_=xd[56 + s:57 + s, :])
                    Cb = pC.tile([128, L], BF, tag="Cb", name=f"Cb{dk}{s}")
                    nc.gpsimd.partition_broadcast(Cb[:, :], sC[0:1, :])
                    for h in range(2):
                        g = h * 48 + s
                        dA = pAM.tile([128, L], BF, tag="AM",
                                      name=f"dA{dk}{s}{h}")
                        nc.scalar.activation(dA[:, :], t_dt[(dk, h)][:, :],
                                             AF.Exp, scale=A_sb[dk][:, g:g + 1])
                        Xt = pX.tile([128, L], BF, tag="X", name=f"X{dk}{s}{h}")
                        eng = nc.gpsimd if X_ON_GP else nc.vector
                        eng.tensor_tensor(Xt[:, :], t_dtu[(dk, h)][:, :],
                                          Bb[:, :], op=OP.mult)
                        Ht = pH.tile([128, L], BF, tag="H", name=f"H{dk}{s}{h}")
                        if dk == "f":
                            nc.vector.tensor_tensor_scan(
                                Ht[:, :], dA[:, :], Xt[:, :], 0.0,
                                OP.mult, OP.add)
                        else:
                            nc.vector.tensor_tensor_scan(
                                rev(Ht), rev(dA), rev(Xt), 0.0,
                                OP.mult, OP.add)
                        Mt = pAM.tile([128, L], BF, tag="AM",
                                      name=f"M{dk}{s}{h}")
                        nc.vector.tensor_tensor(Mt[:, :], Ht[:, :], Cb[:, :],
                                                op=OP.mult)
                        for c in range(NCH):
                            nc.tensor.matmul(p_y[h][:, C512[c]], ident_sb[:, :],
                                             Mt[:, C512[c]],
                                             start=(s == 0), stop=False)
                # fold u*Dp in as the final accumulated matmul (lhsT=diag(Dp))
                for h in range(2):
                    for c in range(NCH):
                        nc.tensor.matmul(p_y[h][:, C512[c]],
                                         dpd[dk][:, h * 128:(h + 1) * 128],
                                         t_u[(dk, h)][:, C512[c]],
                                         start=False, stop=True)
                if dk == "f":
                    for h in range(2):
                        yf = pers.tile([128, L], F32, tag=f"yf{h}",
                                       name=f"yf{h}")
                        nc.vector.tensor_copy(yf[:, :], p_y[h][:, :])
                        t_yf[h] = yf
                else:
                    # combine: yg = (yf + yb) * sz   (bf16)
                    for h in range(2):
                        yfb = pX.tile([128, L], BF, tag="X", name=f"yfb{h}")
                        nc.vector.tensor_tensor(yfb[:, :], t_yf[h][:, :],
                                                p_y[h][:, :], op=OP.add)
                        yg = pH.tile([128, L], BF, tag="H", name=f"yg{h}")
                        nc.vector.tensor_tensor(yg[:, :], yfb[:, :],
                                                t_sz[h][:, :], op=OP.mult)
                        t_yg[h] = yg

            # ---------------- out proj + residual -------------------------
            t_out = bigf.tile([128, L], F32, tag="bigf", name="outsb")
            for c in range(NCH):
                po = ps.tile([128, 512], F32, tag="ps", name=f"po{c}")
                for h in range(2):
                    nc.tensor.matmul(po[:, :], wout_sb[:, h * 128:(h + 1) * 128],
                                     t_yg[h][:, C512[c]],
                                     start=(h == 0), stop=(h == 1))
                nc.vector.tensor_tensor(t_out[:, C512[c]], po[:, :],
                                        t_x[:, C512[c]], op=OP.add)
            nc.gpsimd.dma_start(out=d_out[:, :], in_=t_out[:, :])

    nc.compile()
    return nc


# ---------------------------------------------------------------- host -----
def _prep_weights(inputs):
    import ml_dtypes
    f = np.float32
    bf = ml_dtypes.bfloat16
    ip = inputs["in_proj_w"].astype(f)          # (512, 128)
    xc_w = ip[:D_INNER]                          # (256, 128)
    z_w = ip[D_INNER:]                           # (256, 128)

    def conv_fold(conv_w):
        # wci[p, (k*2+h)*128 + m] = conv_w[h*128+m, 0, k] * xc_w[h*128+m, p]
        out = np.zeros((128, 1024), f)
        for k in range(4):
            for h in range(2):
                blk = (conv_w[h * 128:(h + 1) * 128, 0, k][:, None]
                       * xc_w[h * 128:(h + 1) * 128, :])   # (128m, 128p)
                out[:, (k * 2 + h) * 128:(k * 2 + h + 1) * 128] = blk.T
        return out

    def xproj_pack(w):                           # w: (104, 256)
        out = np.zeros((128, 208), f)
        for h in range(2):
            out[:, h * 104:(h + 1) * 104] = w[:, h * 128:(h + 1) * 128].T
        return out.astype(bf)

    wz = np.zeros((128, 256), f)
    for h in range(2):
        wz[:, h * 128:(h + 1) * 128] = z_w[h * 128:(h + 1) * 128, :].T
    wout = np.zeros((128, 256), f)
    op = inputs["out_proj_w"].astype(f)          # (128, 256)
    for h in range(2):
        wout[:, h * 128:(h + 1) * 128] = op[:, h * 128:(h + 1) * 128].T

    def A_pack(A_log):
        A = -np.exp(A_log.astype(f))             # (256, 48)
        out = np.zeros((128, 96), f)
        for h in range(2):
            out[:, h * 48:(h + 1) * 48] = A[h * 128:(h + 1) * 128, :]
        return out

    def halves(v):                               # (256,) -> (128, 2)
        return np.stack([v[:128], v[128:]], axis=1).astype(f)

    def dp_diag(v):                              # (256,) -> [128, 256] bf16
        out = np.zeros((128, 256), f)
        out[:, 0:128] = np.diag(v[:128])
        out[:, 128:256] = np.diag(v[128:])
        return out.astype(bf)

    cols = np.zeros((128, 12), f)
    cols[:, 0] = inputs["ln_w"]
    cols[:, 1] = inputs["ln_b"]
    cols[:, 2:4] = halves(inputs["dt_proj_b"])
    cols[:, 4:6] = halves(inputs["dt_proj_b_b"])
    cols[:, 6:8] = halves(inputs["conv_b"])
    cols[:, 8:10] = halves(inputs["conv_b_b"])
    cols[:, 10] = EPS
    cols[:, 11] = 1.0

    w = {
        "wci_f": conv_fold(inputs["conv_w"].astype(f)),
        "wci_b": conv_fold(inputs["conv_w_b"].astype(f)),
        "wz": wz,
        "wx_f": xproj_pack(inputs["x_proj_w"].astype(f)),
        "wx_b": xproj_pack(inputs["x_proj_w_b"].astype(f)),
        "wdt_f": inputs["dt_proj_w"].astype(f).T.copy().astype(bf),  # (8,256)
        "wdt_b": inputs["dt_proj_w_b"].astype(f).T.copy().astype(bf),
        "wout": wout.astype(bf),
        "A_f": A_pack(inputs["A_log"]),
        "A_b": A_pack(inputs["A_b_log"]),
        "cols": cols,
        "ones128": np.full((128, 128), 1.0 / 128.0, f),
        "ones_1x128": np.ones((1, 128), f),
        "identv": np.eye(128).astype(bf),
        "dpd_f": dp_diag(inputs["Dp"].astype(f)),
        "dpd_b": dp_diag(inputs["Dp_b"].astype(f)),
    }
    return w


def _get_nc():
    if "nc" not in _CACHE:
        _CACHE["nc"] = _build_nc()
    return _CACHE["nc"]


def _make_runner():
    """Build a cached jitted 8-core runner (mirrors run_bass_via_pjrt)."""
    import jax
    from jax.sharding import Mesh, PartitionSpec
    from jax.experimental.shard_map import shard_map
    from concourse import bass2jax, mybir

    nc = _get_nc()
    bass2jax.install_neuronx_cc_hook()
    part_name = nc.partition_id_tensor.name if nc.partition_id_tensor else None
    in_names, out_names, out_avals, zero_shapes = [], [], [], []
    for alloc in nc.m.functions[0].allocations:
        if not isinstance(alloc, mybir.MemoryLocationSet):
            continue
        name = alloc.memorylocations[0].name
        if alloc.kind == "ExternalInput":
            if name != part_name:
                in_names.append(name)
        elif alloc.kind == "ExternalOutput":
            out_names.append(name)
            shape = tuple(alloc.tensor_shape)
            dtype = mybir.dt.np(alloc.dtype)
            out_avals.append(jax.core.ShapedArray(shape, dtype))
            zero_shapes.append((shape, dtype))
    n_params = len(in_names)
    all_names = in_names + out_names
    if part_name is not None:
        all_names = all_names + [part_name]
    donate = tuple(range(n_params, n_params + len(out_names)))

    def _body(*args):
        operands = list(args)
        if part_name is not None:
            operands.append(bass2jax.partition_id_tensor())
        outs = bass2jax._bass_exec_p.bind(
            *operands,
            out_avals=tuple(out_avals),
            in_names=tuple(all_names),
            out_names=tuple(out_names),
            lowering_input_output_aliases=(),
            sim_require_finite=True,
            sim_require_nnan=True,
            nc=nc,
        )
        return tuple(outs)

    devices = jax.devices()[:N_CORES]
    mesh = Mesh(np.asarray(devices), ("core",))
    n_args = n_params + len(out_names)
    sharded = jax.jit(
        shard_map(_body, mesh=mesh,
                  in_specs=(PartitionSpec("core"),) * n_args,
                  out_specs=(PartitionSpec("core"),) * len(out_names),
                  check_rep=False),
        donate_argnums=donate, keep_unused=True)

    def run(in_maps):
        concat_in = [
            np.concatenate([np.asarray(in_maps[c][nm]) for c in range(N_CORES)],
                           axis=0)
            for nm in in_names
        ]
        concat_zeros = [np.zeros((N_CORES * sh[0], *sh[1:]), dt)
                        for sh, dt in zero_shapes]
        out_arrs = sharded(*concat_in, *concat_zeros)
        out_arrs = [np.asarray(a) for a in out_arrs]
        return [
            {nm: out_arrs[i].reshape(N_CORES, *out_avals[i].shape)[c]
             for i, nm in enumerate(out_names)}
            for c in range(N_CORES)
        ]

    return run


def _run(in_maps, n_iter=1):
    """Execute on 8 cores; returns (results, best_wall_ns)."""
    if "runner" not in _CACHE:
        _CACHE["runner"] = _make_runner()
    run = _CACHE["runner"]
    best = None
    results = None
    for _ in range(n_iter):
        t0 = time.perf_counter_ns()
        results = run(in_maps)
        dt_ns = time.perf_counter_ns() - t0
        best = dt_ns if best is None else min(best, dt_ns)
    return results, best


def kernel(**inputs):
    inputs = {k: np.asarray(v) for k, v in inputs.items()}
    w = _prep_weights(inputs)
    x = inputs["x"].astype(np.float32)           # (8, 2048, 128)
    in_maps = []
    for b in range(N_CORES):
        m = dict(w)
        m["x_T"] = np.ascontiguousarray(x[b].T)  # (128, 2048)
        in_maps.append(m)
    results, wall = _run(in_maps, n_iter=1)
    _CACHE["last_wall_ns"] = wall
    out = np.stack([results[b]["out_T"].T for b in range(N_CORES)])
    return out.astype(np.float32)


# revision 19
# speedup vs baseline: 1.5201x; 1.0302x over previous
"""BiMamba block Trainium2 kernel.

Contract: kernel(**inputs) takes FULL inputs (shapes from the problem spec),
returns the FULL (8, 2048, 128) float32 output. Internally shards
data-parallel over batch across 8 NeuronCores and runs a Bass/Tile kernel.

Layout per core (batch element b):
  - everything lives as [128 partitions, L free] tiles; d_inner=256 is split
    into two halves of 128 channels.
  - LayerNorm via PE column-sum matmuls (mean/var broadcast through PSUM).
  - causal depthwise conv4 folded into the in_proj matmul: 4 shifted
    PSUM-accumulated matmuls with weights conv_w[d,k] * in_proj_w[d,:].
  - softplus(x) = Ln(Exp(x) + 1) on ACT (same table set as the scan's exps).
  - selective scan: per (half, state) group g the recurrence
    h[l] = exp(dt*A_g)*h[l-1] + (dt*u)*B runs as one hardware
    tensor_tensor_scan along the free dim (fp32 internal state, bf16 data).
  - y = sum_s C_s * h_s accumulated in PSUM via identity-matmul on PE;
    u*Dp folded in as one extra accumulated matmul with lhsT=diag(Dp).
  - backward direction: same code with negative-stride (reversed) APs into
    the scan; conv uses right-shifted taps. No explicit flips.
"""
import sys
import time

sys.path.insert(0, "/opt/trn_rl_repo")

import numpy as np

B_SZ, L, D_MODEL = 8, 2048, 128
D_STATE, D_CONV = 48, 4
D_INNER = 256
DT_RANK = 8
N_CORES = 8
EPS = 1e-5
NCH = 4  # 512-column chunks per L

X_ON_GP = True    # X = dtu*Bb multiply on GPSIMD (else DVE)

_CACHE = {}


# ---------------------------------------------------------------- device ---
def _build_nc():
    import concourse.bass as bass
    import concourse.bacc as bacc
    import concourse.tile as tile
    from concourse import mybir
    from contextlib import ExitStack

    F32 = mybir.dt.float32
    BF = mybir.dt.bfloat16
    AF = mybir.ActivationFunctionType
    OP = mybir.AluOpType

    nc = bacc.Bacc("TRN2", target_bir_lowering=False, debug=False)

    def din(name, shape, dt=F32):
        return nc.dram_tensor(name, shape, dt, kind="ExternalInput")

    d_x = din("x_T", [128, L])
    d_wci = {"f": din("wci_f", [128, 1024]), "b": din("wci_b", [128, 1024])}
    d_wz = din("wz", [128, 256])
    d_wx = {"f": din("wx_f", [128, 208], BF), "b": din("wx_b", [128, 208], BF)}
    d_wdt = {"f": din("wdt_f", [8, 256], BF), "b": din("wdt_b", [8, 256], BF)}
    d_wout = din("wout", [128, 256], BF)
    d_A = {"f": din("A_f", [128, 96]), "b": din("A_b", [128, 96])}
    # cols: ln_w, ln_b, dtb_f(2), dtb_b(2), cb_f(2), cb_b(2), eps, one
    d_cols = din("cols", [128, 12])
    d_ones = din("ones128", [128, 128])      # all 1/128
    d_ones1 = din("ones_1x128", [1, 128])    # all 1.0
    d_ident = din("identv", [128, 128], BF)  # identity
    d_dpd = {"f": din("dpd_f", [128, 256], BF),  # diag(Dp) halves
             "b": din("dpd_b", [128, 256], BF)}
    d_out = nc.dram_tensor("out_T", [128, L], F32, kind="ExternalOutput")
    d_xdr = {"f": nc.dram_tensor("xdr_f", [96, L], BF),
             "b": nc.dram_tensor("xdr_b", [96, L], BF)}

    C512 = [slice(c * 512, (c + 1) * 512) for c in range(NCH)]

    with tile.TileContext(nc) as tc:
        with ExitStack() as ctx:
            consts = ctx.enter_context(tc.tile_pool(name="consts", bufs=1))
            pers = ctx.enter_context(tc.tile_pool(name="pers", bufs=1))
            bigf = ctx.enter_context(tc.tile_pool(name="bigf", bufs=2))
            v512 = ctx.enter_context(tc.tile_pool(name="v512", bufs=8))
            pAM = ctx.enter_context(tc.tile_pool(name="pAM", bufs=4))
            pX = ctx.enter_context(tc.tile_pool(name="pX", bufs=3))
            pH = ctx.enter_context(tc.tile_pool(name="pH", bufs=3))
            pB = ctx.enter_context(tc.tile_pool(name="pB", bufs=3))
            pC = ctx.enter_context(tc.tile_pool(name="pC", bufs=3))
            ps = ctx.enter_context(tc.tile_pool(name="ps", bufs=2, space="PSUM"))

            def cload(d, shape, tag, dt=F32):
                t = consts.tile(shape, dt, tag=tag, name=f"c_{tag}")
                nc.sync.dma_start(out=t[:, :], in_=d[:, :])
                return t

            ones_sb = cload(d_ones, [128, 128], "ones")
            ones1_sb = cload(d_ones1, [1, 128], "ones1")
            ident_sb = cload(d_ident, [128, 128], "ident", BF)
            wci = {k: cload(d_wci[k], [128, 1024], f"wci_{k}") for k in "fb"}
            wz_sb = cload(d_wz, [128, 256], "wz")
            wx = {k: cload(d_wx[k], [128, 208], f"wx_{k}", BF) for k in "fb"}
            wdt = {k: cload(d_wdt[k], [8, 256], f"wdt_{k}", BF) for k in "fb"}
            wout_sb = cload(d_wout, [128, 256], "wout", BF)
            A_sb = {k: cload(d_A[k], [128, 96], f"A_{k}") for k in "fb"}
            dpd = {k: cload(d_dpd[k], [128, 256], f"dpd_{k}", BF) for k in "fb"}
            cols = cload(d_cols, [128, 12], "cols")
            lnw, lnb = cols[:, 0:1], cols[:, 1:2]
            dtb = {"f": (cols[:, 2:3], cols[:, 3:4]),
                   "b": (cols[:, 4:5], cols[:, 5:6])}
            cb = {"f": (cols[:, 6:7], cols[:, 7:8]),
                  "b": (cols[:, 8:9], cols[:, 9:10])}
            c_eps, c_one = cols[:, 10:11], cols[:, 11:12]

            # ---------------- LayerNorm (D in partitions, stats via PE) ----
            t_x = pers.tile([128, L], F32, tag="x")
            nc.sync.dma_start(out=t_x[:, :], in_=d_x[:, :])

            p_mu = ps.tile([128, L], F32, tag="ps", name="p_mu")
            for c in range(NCH):
                nc.tensor.matmul(p_mu[:, C512[c]], ones_sb[:, :],
                                 t_x[:, C512[c]], start=True, stop=True)
            t_xc = bigf.tile([128, L], F32, tag="bigf", name="xcen")
            nc.vector.tensor_tensor(t_xc[:, :], t_x[:, :], p_mu[:, :],
                                    op=OP.subtract)
            t_sq = bigf.tile([128, L], F32, tag="bigf", name="sq")
            nc.scalar.square(t_sq[:, :], t_xc[:, :])
            p_var = ps.tile([128, L], F32, tag="ps", name="p_var")
            for c in range(NCH):
                nc.tensor.matmul(p_var[0:1, C512[c]], ones_sb[:, 0:1],
                                 t_sq[:, C512[c]], start=True, stop=True)
            p_rb = ps.tile([128, L], F32, tag="ps", name="p_rb")
            for c in range(NCH):
                lnr = v512.tile([1, 512], F32, tag="v512", name=f"lnr{c}")
                nc.scalar.activation(lnr[0:1, :], p_var[0:1, C512[c]], AF.Ln,
                                     bias=c_eps[0:1, :])
                rst = v512.tile([1, 512], F32, tag="v512", name=f"rst{c}")
                nc.scalar.activation(rst[0:1, :], lnr[0:1, :], AF.Exp,
                                     scale=-0.5)
                nc.tensor.matmul(p_rb[:, C512[c]], ones1_sb[:, :],
                                 rst[0:1, :], start=True, stop=True)
            t_xnp = pers.tile([128, L + 6], F32, tag="xnpad")
            nc.vector.memset(t_xnp[:, 0:3], 0.0)
            nc.vector.memset(t_xnp[:, L + 3:L + 6], 0.0)
            t_xn0 = bigf.tile([128, L], F32, tag="bigf", name="xn0")
            nc.vector.tensor_tensor(t_xn0[:, :], t_xc[:, :], p_rb[:, :],
                                    op=OP.mult)
            nc.scalar.activation(t_xnp[:, 3:L + 3], t_xn0[:, :], AF.Identity,
                                 bias=lnb, scale=lnw)

            # ---------------- projections (per direction) -----------------
            # conv-folded in_proj -> silu -> u (bf16), chunked through PSUM.
            # fwd taps read xn shifted by k-3 (left pad), bwd by 3-k (right).
            t_u, t_sz, t_xd, t_dt, t_dtu = {}, {}, {}, {}, {}

            def emit_prep(dk):
                for h in range(2):
                    u = pers.tile([128, L], BF, tag=f"u_{dk}{h}",
                                  name=f"u_{dk}{h}")
                    t_u[(dk, h)] = u
                    pu = ps.tile([128, L], F32, tag="ps", name=f"pu{dk}{h}")
                    for c in range(NCH):
                        for k in range(4):
                            off = (c * 512 + k) if dk == "f" else (c * 512 + 6 - k)
                            nc.tensor.matmul(
                                pu[:, C512[c]],
                                wci[dk][:, (k * 2 + h) * 128:(k * 2 + h + 1) * 128],
                                t_xnp[:, off:off + 512],
                                start=(k == 0), stop=(k == 3))
                        uc = v512.tile([128, 512], F32, tag="v512",
                                       name=f"uc{dk}{h}{c}")
                        nc.scalar.activation(uc[:, :], pu[:, C512[c]],
                                             AF.Identity, bias=cb[dk][h])
                        sg = v512.tile([128, 512], F32, tag="v512",
                                       name=f"sg{dk}{h}{c}")
                        nc.scalar.activation(sg[:, :], uc[:, :], AF.Sigmoid)
                        nc.vector.tensor_tensor(u[:, C512[c]], uc[:, :],
                                                sg[:, :], op=OP.mult)
                # x_proj -> xd (bf16): rows 0:8 dt_raw, 8:56 B, 56:104 C
                xd = pers.tile([104, L], BF, tag=f"xd_{dk}", name=f"xd_{dk}")
                px = ps.tile([128, L], F32, tag="ps", name=f"px{dk}")
                for c in range(NCH):
                    for h in range(2):
                        nc.tensor.matmul(px[0:104, C512[c]],
                                         wx[dk][:, h * 104:(h + 1) * 104],
                                         t_u[(dk, h)][:, C512[c]],
                                         start=(h == 0), stop=(h == 1))
                    nc.scalar.copy(xd[:, C512[c]], px[0:104, C512[c]])
                nc.sync.dma_start(out=d_xdr[dk][:, :], in_=xd[8:104, :])
                t_xd[dk] = xd
                # dt = softplus(dt_raw @ wdt.T + dtb) = Ln(Exp(.) + 1), bf16
                for h in range(2):
                    dt_t = pers.tile([128, L], BF, tag=f"dt_{dk}{h}",
                                     name=f"dt_{dk}{h}")
                    pd = ps.tile([128, L], F32, tag="ps", name=f"pd{dk}{h}")
                    for c in range(NCH):
                        nc.tensor.matmul(pd[:, C512[c]],
                                         wdt[dk][:, h * 128:(h + 1) * 128],
                                         t_xd[dk][0:8, C512[c]],
                                         start=True, stop=True)
                        te = v512.tile([128, 512], F32, tag="v512",
                                       name=f"te{dk}{h}{c}")
                        nc.scalar.activation(te[:, :], pd[:, C512[c]], AF.Exp,
                                             bias=dtb[dk][h])
                        nc.scalar.activation(dt_t[:, C512[c]], te[:, :],
                                             AF.Ln, bias=c_one)
                    t_dt[(dk, h)] = dt_t
                for h in range(2):
                    dtu = pers.tile([128, L], BF, tag=f"dtu_{dk}{h}",
                                    name=f"dtu_{dk}{h}")
                    nc.vector.tensor_tensor(dtu[:, :], t_dt[(dk, h)][:, :],
                                            t_u[(dk, h)][:, :], op=OP.mult)
                    t_dtu[(dk, h)] = dtu

            def emit_z():
                for h in range(2):
                    sz = pers.tile([128, L], BF, tag=f"sz{h}", name=f"sz{h}")
                    t_sz[h] = sz
                    pz = ps.tile([128, L], F32, tag="ps", name=f"pz{h}")
                    for c in range(NCH):
                        nc.tensor.matmul(pz[:, C512[c]],
                                         wz_sb[:, h * 128:(h + 1) * 128],
                                         t_xnp[:, 3 + c * 512:3 + (c + 1) * 512],
                                         start=True, stop=True)
                        zc = v512.tile([128, 512], F32, tag="v512",
                                       name=f"zc{h}{c}")
                        nc.scalar.copy(zc[:, :], pz[:, C512[c]])
                        sg = v512.tile([128, 512], F32, tag="v512",
                                       name=f"sgz{h}{c}")
                        nc.scalar.activation(sg[:, :], zc[:, :], AF.Sigmoid)
                        nc.vector.tensor_tensor(sz[:, C512[c]], zc[:, :],
                                                sg[:, :], op=OP.mult)

            # ---------------- scan loops ---------------------------------
            def rev(ap):
                return ap[:, ::-1]

            import os as _os
            _phases = _os.environ.get("BIMAMBA_PHASES", "all")
            t_yf = {}
            t_yg = {}
            dirs = {"prep": "", "fwd": "f", "all": "fb"}[_phases]
            emit_prep("f")
            emit_prep("b")
            emit_z()
            for dk in dirs:
                p_y = {}
                for h in range(2):
                    p_y[h] = ps.tile([128, L], F32, tag="ps",
                                     name=f"py_{dk}{h}")
                xd = t_xd[dk]
                def bcast_ap(row):
                    r = d_xdr[dk][row:row + 1, :]
                    return bass.AP(tensor=r.tensor, offset=r.offset,
                                   ap=[[0, 128]] + list(r.ap[1:]))

                for s in range(D_STATE):
                    Bb = pB.tile([128, L], BF, tag="Bb", name=f"Bb{dk}{s}")
                    nc.sync.dma_start(out=Bb[:, :], in_=bcast_ap(s))
                    Cb = pC.tile([128, L], BF, tag="Cb", name=f"Cb{dk}{s}")
                    nc.sync.dma_start(out=Cb[:, :], in_=bcast_ap(48 + s))
                    for h in range(2):
                        g = h * 48 + s
                        dA = pAM.tile([128, L], BF, tag="AM",
                                      name=f"dA{dk}{s}{h}")
                        nc.scalar.activation(dA[:, :], t_dt[(dk, h)][:, :],
                                             AF.Exp, scale=A_sb[dk][:, g:g + 1])
                        Xt = pX.tile([128, L], BF, tag="X", name=f"X{dk}{s}{h}")
                        eng = nc.vector if (s * 2 + h) % 7 == 0 else nc.gpsimd
                        eng.tensor_tensor(Xt[:, :], t_dtu[(dk, h)][:, :],
                                          Bb[:, :], op=OP.mult)
                        Ht = pH.tile([128, L], BF, tag="H", name=f"H{dk}{s}{h}")
                        if dk == "f":
                            nc.vector.tensor_tensor_scan(
                                Ht[:, :], dA[:, :], Xt[:, :], 0.0,
                                OP.mult, OP.add)
                        else:
                            nc.vector.tensor_tensor_scan(
                                rev(Ht), rev(dA), rev(Xt), 0.0,
                                OP.mult, OP.add)
                        Mt = pAM.tile([128, L], BF, tag="AM",
                                      name=f"M{dk}{s}{h}")
                        nc.vector.tensor_tensor(Mt[:, :], Ht[:, :], Cb[:, :],
                                                op=OP.mult)
                        for c in range(NCH):
                            nc.tensor.matmul(p_y[h][:, C512[c]], ident_sb[:, :],
                                             Mt[:, C512[c]],
                                             start=(s == 0), stop=False)
                # fold u*Dp in as the final accumulated matmul (lhsT=diag(Dp))
                for h in range(2):
                    for c in range(NCH):
                        nc.tensor.matmul(p_y[h][:, C512[c]],
                                         dpd[dk][:, h * 128:(h + 1) * 128],
                                         t_u[(dk, h)][:, C512[c]],
                                         start=False, stop=True)
                if dk == "f":
                    for h in range(2):
                        yf = pers.tile([128, L], F32, tag=f"yf{h}",
                                       name=f"yf{h}")
                        nc.vector.tensor_copy(yf[:, :], p_y[h][:, :])
                        t_yf[h] = yf
                else:
                    # combine: yg = (yf + yb) * sz   (bf16)
                    for h in range(2):
                        yfb = pX.tile([128, L], BF, tag="X", name=f"yfb{h}")
                        nc.vector.tensor_tensor(yfb[:, :], t_yf[h][:, :],
                                                p_y[h][:, :], op=OP.add)
                        yg = pH.tile([128, L], BF, tag="H", name=f"yg{h}")
                        nc.vector.tensor_tensor(yg[:, :], yfb[:, :],
                                                t_sz[h][:, :], op=OP.mult)
                        t_yg[h] = yg

            # ---------------- out proj + residual -------------------------
            t_out = bigf.tile([128, L], F32, tag="bigf", name="outsb")
            if _phases != "all":
                t_yg = {h: t_u[("f", h)] for h in range(2)}  # timing probe only
            po = ps.tile([128, L], F32, tag="ps", name="po")
            for c in range(NCH):
                for h in range(2):
                    nc.tensor.matmul(po[:, C512[c]], wout_sb[:, h * 128:(h + 1) * 128],
                                     t_yg[h][:, C512[c]],
                                     start=(h == 0), stop=(h == 1))
                nc.vector.tensor_tensor(t_out[:, C512[c]], po[:, C512[c]],
                                        t_x[:, C512[c]], op=OP.add)
            nc.sync.dma_start(out=d_out[:, :], in_=t_out[:, :])

    nc.compile()
    return nc


# ---------------------------------------------------------------- host -----
def _prep_weights(inputs):
    import ml_dtypes
    f = np.float32
    bf = ml_dtypes.bfloat16
    ip = inputs["in_proj_w"].astype(f)          # (512, 128)
    xc_w = ip[:D_INNER]                          # (256, 128)
    z_w = ip[D_INNER:]                           # (256, 128)

    def conv_fold(conv_w):
        # wci[p, (k*2+h)*128 + m] = conv_w[h*128+m, 0, k] * xc_w[h*128+m, p]
        out = np.zeros((128, 1024), f)
        for k in range(4):
            for h in range(2):
                blk = (conv_w[h * 128:(h + 1) * 128, 0, k][:, None]
                       * xc_w[h * 128:(h + 1) * 128, :])   # (128m, 128p)
                out[:, (k * 2 + h) * 128:(k * 2 + h + 1) * 128] = blk.T
        return out

    def xproj_pack(w):                           # w: (104, 256)
        out = np.zeros((128, 208), f)
        for h in range(2):
            out[:, h * 104:(h + 1) * 104] = w[:, h * 128:(h + 1) * 128].T
        return out.astype(bf)

    wz = np.zeros((128, 256), f)
    for h in range(2):
        wz[:, h * 128:(h + 1) * 128] = z_w[h * 128:(h + 1) * 128, :].T
    wout = np.zeros((128, 256), f)
    op = inputs["out_proj_w"].astype(f)          # (128, 256)
    for h in range(2):
        wout[:, h * 128:(h + 1) * 128] = op[:, h * 128:(h + 1) * 128].T

    def A_pack(A_log):
        A = -np.exp(A_log.astype(f))             # (256, 48)
        out = np.zeros((128, 96), f)
        for h in range(2):
            out[:, h * 48:(h + 1) * 48] = A[h * 128:(h + 1) * 128, :]
        return out

    def halves(v):                               # (256,) -> (128, 2)
        return np.stack([v[:128], v[128:]], axis=1).astype(f)

    def dp_diag(v):                              # (256,) -> [128, 256] bf16
        out = np.zeros((128, 256), f)
        out[:, 0:128] = np.diag(v[:128])
        out[:, 128:256] = np.diag(v[128:])
        return out.astype(bf)

    cols = np.zeros((128, 12), f)
    cols[:, 0] = inputs["ln_w"]
    cols[:, 1] = inputs["ln_b"]
    cols[:, 2:4] = halves(inputs["dt_proj_b"])
    cols[:, 4:6] = halves(inputs["dt_proj_b_b"])
    cols[:, 6:8] = halves(inputs["conv_b"])
    cols[:, 8:10] = halves(inputs["conv_b_b"])
    cols[:, 10] = EPS
    cols[:, 11] = 1.0

    w = {
        "wci_f": conv_fold(inputs["conv_w"].astype(f)),
        "wci_b": conv_fold(inputs["conv_w_b"].astype(f)),
        "wz": wz,
        "wx_f": xproj_pack(inputs["x_proj_w"].astype(f)),
        "wx_b": xproj_pack(inputs["x_proj_w_b"].astype(f)),
        "wdt_f": inputs["dt_proj_w"].astype(f).T.copy().astype(bf),  # (8,256)
        "wdt_b": inputs["dt_proj_w_b"].astype(f).T.copy().astype(bf),
        "wout": wout.astype(bf),
        "A_f": A_pack(inputs["A_log"]),
        "A_b": A_pack(inputs["A_b_log"]),
        "cols": cols,
        "ones128": np.full((128, 128), 1.0 / 128.0, f),
        "ones_1x128": np.ones((1, 128), f),
        "identv": np.eye(128).astype(bf),
        "dpd_f": dp_diag(inputs["Dp"].astype(f)),
        "dpd_b": dp_diag(inputs["Dp_b"].astype(f)),
    }
    return w


def _get_nc():
    if "nc" not in _CACHE:
        _CACHE["nc"] = _build_nc()
    return _CACHE["nc"]


def _make_runner():
    """Build a cached jitted 8-core runner (mirrors run_bass_via_pjrt)."""
    import jax
    from jax.sharding import Mesh, PartitionSpec
    from jax.experimental.shard_map import shard_map
    from concourse import bass2jax, mybir

    nc = _get_nc()
    bass2jax.install_neuronx_cc_hook()
    part_name = nc.partition_id_tensor.name if nc.partition_id_tensor else None
    in_names, out_names, out_avals, zero_shapes = [], [], [], []
    for alloc in nc.m.functions[0].allocations:
        if not isinstance(alloc, mybir.MemoryLocationSet):
            continue
        name = alloc.memorylocations[0].name
        if alloc.kind == "ExternalInput":
            if name != part_name:
                in_names.append(name)
        elif alloc.kind == "ExternalOutput":
            out_names.append(name)
            shape = tuple(alloc.tensor_shape)
            dtype = mybir.dt.np(alloc.dtype)
            out_avals.append(jax.core.ShapedArray(shape, dtype))
            zero_shapes.append((shape, dtype))
    n_params = len(in_names)
    all_names = in_names + out_names
    if part_name is not None:
        all_names = all_names + [part_name]
    donate = tuple(range(n_params, n_params + len(out_names)))

    def _body(*args):
        operands = list(args)
        if part_name is not None:
            operands.append(bass2jax.partition_id_tensor())
        outs = bass2jax._bass_exec_p.bind(
            *operands,
            out_avals=tuple(out_avals),
            in_names=tuple(all_names),
            out_names=tuple(out_names),
            lowering_input_output_aliases=(),
            sim_require_finite=True,
            sim_require_nnan=True,
            nc=nc,
        )
        return tuple(outs)

    devices = jax.devices()[:N_CORES]
    mesh = Mesh(np.asarray(devices), ("core",))
    n_args = n_params + len(out_names)
    sharded = jax.jit(
        shard_map(_body, mesh=mesh,
                  in_specs=(PartitionSpec("core"),) * n_args,
                  out_specs=(PartitionSpec("core"),) * len(out_names),
                  check_rep=False),
        donate_argnums=donate, keep_unused=True)

    def run(in_maps):
        concat_in = [
            np.concatenate([np.asarray(in_maps[c][nm]) for c in range(N_CORES)],
                           axis=0)
            for nm in in_names
        ]
        concat_zeros = [np.zeros((N_CORES * sh[0], *sh[1:]), dt)
                        for sh, dt in zero_shapes]
        out_arrs = sharded(*concat_in, *concat_zeros)
        out_arrs = [np.asarray(a) for a in out_arrs]
        return [
            {nm: out_arrs[i].reshape(N_CORES, *out_avals[i].shape)[c]
             for i, nm in enumerate(out_names)}
            for c in range(N_CORES)
        ]

    return run


def _run(in_maps, n_iter=1):
    """Execute on 8 cores; returns (results, best_wall_ns)."""
    if "runner" not in _CACHE:
        _CACHE["runner"] = _make_runner()
    run = _CACHE["runner"]
    best = None
    results = None
    for _ in range(n_iter):
        t0 = time.perf_counter_ns()
        results = run(in_maps)
        dt_ns = time.perf_counter_ns() - t0
        best = dt_ns if best is None else min(best, dt_ns)
    return results, best


def kernel(**inputs):
    inputs = {k: np.asarray(v) for k, v in inputs.items()}
    w = _prep_weights(inputs)
    x = inputs["x"].astype(np.float32)           # (8, 2048, 128)
    in_maps = []
    for b in range(N_CORES):
        m = dict(w)
        m["x_T"] = np.ascontiguousarray(x[b].T)  # (128, 2048)
        in_maps.append(m)
    results, wall = _run(in_maps, n_iter=1)
    _CACHE["last_wall_ns"] = wall
    out = np.stack([results[b]["out_T"].T for b in range(N_CORES)])
    return out.astype(np.float32)


# revision 21
# speedup vs baseline: 13.8753x; 9.1281x over previous
"""BiMamba block Trainium2 kernel.

Contract: kernel(**inputs) takes FULL inputs (shapes from the problem spec),
returns the FULL (8, 2048, 128) float32 output. Internally shards
data-parallel over batch across 8 NeuronCores and runs a Bass/Tile kernel.

Layout per core (batch element b):
  - everything lives as [128 partitions, L free] tiles; d_inner=256 is split
    into two halves of 128 channels.
  - LayerNorm via PE column-sum matmuls (mean/var broadcast through PSUM).
  - causal depthwise conv4 folded into the in_proj matmul: 4 shifted
    PSUM-accumulated matmuls with weights conv_w[d,k] * in_proj_w[d,:].
  - softplus(x) = Ln(Exp(x) + 1) on ACT (same table set as the scan's exps).
  - selective scan: per (half, state) group g the recurrence
    h[l] = exp(dt*A_g)*h[l-1] + (dt*u)*B runs as one hardware
    tensor_tensor_scan along the free dim (fp32 internal state, bf16 data).
  - y = sum_s C_s * h_s accumulated in PSUM via identity-matmul on PE;
    u*Dp folded in as one extra accumulated matmul with lhsT=diag(Dp).
  - backward direction: same code with negative-stride (reversed) APs into
    the scan; conv uses right-shifted taps. No explicit flips.
"""
import sys
import time

sys.path.insert(0, "/opt/trn_rl_repo")

import numpy as np

B_SZ, L, D_MODEL = 8, 2048, 128
D_STATE, D_CONV = 48, 4
D_INNER = 256
DT_RANK = 8
N_CORES = 8
EPS = 1e-5
NCH = 4  # 512-column chunks per L

X_ON_GP = True    # X = dtu*Bb multiply on GPSIMD (else DVE)

_CACHE = {}


# ---------------------------------------------------------------- device ---
def _build_nc():
    import concourse.bass as bass
    import concourse.bacc as bacc
    import concourse.tile as tile
    from concourse import mybir
    from contextlib import ExitStack

    F32 = mybir.dt.float32
    BF = mybir.dt.bfloat16
    AF = mybir.ActivationFunctionType
    OP = mybir.AluOpType

    nc = bacc.Bacc("TRN2", target_bir_lowering=False, debug=False)

    def din(name, shape, dt=F32):
        return nc.dram_tensor(name, shape, dt, kind="ExternalInput")

    d_x = din("x_T", [128, L])
    d_wci = {"f": din("wci_f", [128, 1024]), "b": din("wci_b", [128, 1024])}
    d_wz = din("wz", [128, 256])
    d_wx = {"f": din("wx_f", [128, 208], BF), "b": din("wx_b", [128, 208], BF)}
    d_wdt = {"f": din("wdt_f", [8, 256], BF), "b": din("wdt_b", [8, 256], BF)}
    d_wout = din("wout", [128, 256], BF)
    d_A = {"f": din("A_f", [128, 96]), "b": din("A_b", [128, 96])}
    # cols: ln_w, ln_b, dtb_f(2), dtb_b(2), cb_f(2), cb_b(2), eps, one
    d_cols = din("cols", [128, 12])
    d_ones = din("ones128", [128, 128])      # all 1/128
    d_ones1 = din("ones_1x128", [1, 128])    # all 1.0
    d_ident = din("identv", [128, 128], BF)  # identity
    d_dpd = {"f": din("dpd_f", [128, 256], BF),  # diag(Dp) halves
             "b": din("dpd_b", [128, 256], BF)}
    d_out = nc.dram_tensor("out_T", [128, L], F32, kind="ExternalOutput")
    d_xdr = {"f": nc.dram_tensor("xdr_f", [96, L], BF),
             "b": nc.dram_tensor("xdr_b", [96, L], BF)}

    C512 = [slice(c * 512, (c + 1) * 512) for c in range(NCH)]

    with tile.TileContext(nc) as tc:
        with ExitStack() as ctx:
            consts = ctx.enter_context(tc.tile_pool(name="consts", bufs=1))
            pers = ctx.enter_context(tc.tile_pool(name="pers", bufs=1))
            bigf = ctx.enter_context(tc.tile_pool(name="bigf", bufs=2))
            v512 = ctx.enter_context(tc.tile_pool(name="v512", bufs=6))
            pAM = ctx.enter_context(tc.tile_pool(name="pAM", bufs=4))
            pX = ctx.enter_context(tc.tile_pool(name="pX", bufs=3))
            pH = ctx.enter_context(tc.tile_pool(name="pH", bufs=3))
            pB = ctx.enter_context(tc.tile_pool(name="pB", bufs=3))
            pC = ctx.enter_context(tc.tile_pool(name="pC", bufs=3))
            ps = ctx.enter_context(tc.tile_pool(name="ps", bufs=2, space="PSUM"))

            def cload(d, shape, tag, dt=F32):
                t = consts.tile(shape, dt, tag=tag, name=f"c_{tag}")
                nc.sync.dma_start(out=t[:, :], in_=d[:, :])
                return t

            ones_sb = cload(d_ones, [128, 128], "ones")
            ones1_sb = cload(d_ones1, [1, 128], "ones1")
            ident_sb = cload(d_ident, [128, 128], "ident", BF)
            wci = {k: cload(d_wci[k], [128, 1024], f"wci_{k}") for k in "fb"}
            wz_sb = cload(d_wz, [128, 256], "wz")
            wx = {k: cload(d_wx[k], [128, 208], f"wx_{k}", BF) for k in "fb"}
            wdt = {k: cload(d_wdt[k], [8, 256], f"wdt_{k}", BF) for k in "fb"}
            wout_sb = cload(d_wout, [128, 256], "wout", BF)
            A_sb = {k: cload(d_A[k], [128, 96], f"A_{k}") for k in "fb"}
            dpd = {k: cload(d_dpd[k], [128, 256], f"dpd_{k}", BF) for k in "fb"}
            cols = cload(d_cols, [128, 12], "cols")
            lnw, lnb = cols[:, 0:1], cols[:, 1:2]
            dtb = {"f": (cols[:, 2:3], cols[:, 3:4]),
                   "b": (cols[:, 4:5], cols[:, 5:6])}
            cb = {"f": (cols[:, 6:7], cols[:, 7:8]),
                  "b": (cols[:, 8:9], cols[:, 9:10])}
            c_eps, c_one = cols[:, 10:11], cols[:, 11:12]

            # ---------------- LayerNorm (D in partitions, stats via PE) ----
            t_x = pers.tile([128, L], F32, tag="x")
            nc.sync.dma_start(out=t_x[:, :], in_=d_x[:, :])

            p_mu = ps.tile([128, L], F32, tag="ps", name="p_mu")
            for c in range(NCH):
                nc.tensor.matmul(p_mu[:, C512[c]], ones_sb[:, :],
                                 t_x[:, C512[c]], start=True, stop=True)
            t_xc = bigf.tile([128, L], F32, tag="bigf", name="xcen")
            nc.vector.tensor_tensor(t_xc[:, :], t_x[:, :], p_mu[:, :],
                                    op=OP.subtract)
            t_sq = bigf.tile([128, L], F32, tag="bigf", name="sq")
            nc.scalar.square(t_sq[:, :], t_xc[:, :])
            p_var = ps.tile([128, L], F32, tag="ps", name="p_var")
            for c in range(NCH):
                nc.tensor.matmul(p_var[0:1, C512[c]], ones_sb[:, 0:1],
                                 t_sq[:, C512[c]], start=True, stop=True)
            p_rb = ps.tile([128, L], F32, tag="ps", name="p_rb")
            for c in range(NCH):
                lnr = v512.tile([1, 512], F32, tag="v512", name=f"lnr{c}")
                nc.scalar.activation(lnr[0:1, :], p_var[0:1, C512[c]], AF.Ln,
                                     bias=c_eps[0:1, :])
                rst = v512.tile([1, 512], F32, tag="v512", name=f"rst{c}")
                nc.scalar.activation(rst[0:1, :], lnr[0:1, :], AF.Exp,
                                     scale=-0.5)
                nc.tensor.matmul(p_rb[:, C512[c]], ones1_sb[:, :],
                                 rst[0:1, :], start=True, stop=True)
            t_xnp = pers.tile([128, L + 6], F32, tag="xnpad")
            nc.vector.memset(t_xnp[:, 0:3], 0.0)
            nc.vector.memset(t_xnp[:, L + 3:L + 6], 0.0)
            t_xn0 = bigf.tile([128, L], F32, tag="bigf", name="xn0")
            nc.vector.tensor_tensor(t_xn0[:, :], t_xc[:, :], p_rb[:, :],
                                    op=OP.mult)
            nc.scalar.activation(t_xnp[:, 3:L + 3], t_xn0[:, :], AF.Identity,
                                 bias=lnb, scale=lnw)

            # ---------------- projections (per direction) -----------------
            # conv-folded in_proj -> silu -> u (bf16), chunked through PSUM.
            # fwd taps read xn shifted by k-3 (left pad), bwd by 3-k (right).
            t_u, t_sz, t_xd, t_dt, t_dtu = {}, {}, {}, {}, {}

            def emit_prep(dk):
                for h in range(2):
                    u = pers.tile([128, L], BF, tag=f"u_{dk}{h}",
                                  name=f"u_{dk}{h}")
                    t_u[(dk, h)] = u
                    pu = ps.tile([128, L], F32, tag="ps", name=f"pu{dk}{h}")
                    for c in range(NCH):
                        for k in range(4):
                            off = (c * 512 + k) if dk == "f" else (c * 512 + 6 - k)
                            nc.tensor.matmul(
                                pu[:, C512[c]],
                                wci[dk][:, (k * 2 + h) * 128:(k * 2 + h + 1) * 128],
                                t_xnp[:, off:off + 512],
                                start=(k == 0), stop=(k == 3))
                        uc = v512.tile([128, 512], F32, tag="v512",
                                       name=f"uc{dk}{h}{c}")
                        nc.scalar.activation(uc[:, :], pu[:, C512[c]],
                                             AF.Identity, bias=cb[dk][h])
                        sg = v512.tile([128, 512], F32, tag="v512",
                                       name=f"sg{dk}{h}{c}")
                        nc.scalar.activation(sg[:, :], uc[:, :], AF.Sigmoid)
                        nc.vector.tensor_tensor(u[:, C512[c]], uc[:, :],
                                                sg[:, :], op=OP.mult)
                # x_proj -> xd (bf16): rows 0:8 dt_raw, 8:56 B, 56:104 C
                xd = pers.tile([104, L], BF, tag=f"xd_{dk}", name=f"xd_{dk}")
                px = ps.tile([128, L], F32, tag="ps", name=f"px{dk}")
                for c in range(NCH):
                    for h in range(2):
                        nc.tensor.matmul(px[0:104, C512[c]],
                                         wx[dk][:, h * 104:(h + 1) * 104],
                                         t_u[(dk, h)][:, C512[c]],
                                         start=(h == 0), stop=(h == 1))
                    nc.scalar.copy(xd[:, C512[c]], px[0:104, C512[c]])
                nc.sync.dma_start(out=d_xdr[dk][:, :], in_=xd[8:104, :])
                t_xd[dk] = xd
                # dt = softplus(dt_raw @ wdt.T + dtb) = Ln(Exp(.) + 1), bf16
                for h in range(2):
                    dt_t = pers.tile([128, L], BF, tag=f"dt_{dk}{h}",
                                     name=f"dt_{dk}{h}")
                    pd = ps.tile([128, L], F32, tag="ps", name=f"pd{dk}{h}")
                    for c in range(NCH):
                        nc.tensor.matmul(pd[:, C512[c]],
                                         wdt[dk][:, h * 128:(h + 1) * 128],
                                         t_xd[dk][0:8, C512[c]],
                                         start=True, stop=True)
                        te = v512.tile([128, 512], F32, tag="v512",
                                       name=f"te{dk}{h}{c}")
                        nc.scalar.activation(te[:, :], pd[:, C512[c]], AF.Exp,
                                             bias=dtb[dk][h])
                        nc.scalar.activation(dt_t[:, C512[c]], te[:, :],
                                             AF.Ln, bias=c_one)
                    t_dt[(dk, h)] = dt_t
                for h in range(2):
                    dtu = pers.tile([128, L], BF, tag=f"dtu_{dk}{h}",
                                    name=f"dtu_{dk}{h}")
                    nc.vector.tensor_tensor(dtu[:, :], t_dt[(dk, h)][:, :],
                                            t_u[(dk, h)][:, :], op=OP.mult)
                    t_dtu[(dk, h)] = dtu

            def emit_z():
                for h in range(2):
                    sz = pers.tile([128, L], BF, tag=f"sz{h}", name=f"sz{h}")
                    t_sz[h] = sz
                    pz = ps.tile([128, L], F32, tag="ps", name=f"pz{h}")
                    for c in range(NCH):
                        nc.tensor.matmul(pz[:, C512[c]],
                                         wz_sb[:, h * 128:(h + 1) * 128],
                                         t_xnp[:, 3 + c * 512:3 + (c + 1) * 512],
                                         start=True, stop=True)
                        zc = v512.tile([128, 512], F32, tag="v512",
                                       name=f"zc{h}{c}")
                        nc.scalar.copy(zc[:, :], pz[:, C512[c]])
                        sg = v512.tile([128, 512], F32, tag="v512",
                                       name=f"sgz{h}{c}")
                        nc.scalar.activation(sg[:, :], zc[:, :], AF.Sigmoid)
                        nc.vector.tensor_tensor(sz[:, C512[c]], zc[:, :],
                                                sg[:, :], op=OP.mult)

            # ---------------- scan loops ---------------------------------
            def rev(ap):
                return ap[:, ::-1]

            import os as _os
            _phases = _os.environ.get("BIMAMBA_PHASES", "all")
            t_yf = {}
            t_yg = {}
            dirs = {"prep": "", "fwd": "f", "all": "fb"}[_phases]
            emit_prep("f")
            emit_prep("b")
            emit_z()
            for dk in dirs:
                p_y = {}
                for h in range(2):
                    p_y[h] = ps.tile([128, L], F32, tag="ps",
                                     name=f"py_{dk}{h}")
                xd = t_xd[dk]
                def bcast_ap(row):
                    r = d_xdr[dk][row:row + 1, :]
                    return bass.AP(tensor=r.tensor, offset=r.offset,
                                   ap=[[0, 128]] + list(r.ap[1:]))

                for s in range(D_STATE):
                    Bb = pB.tile([128, L], BF, tag="Bb", name=f"Bb{dk}{s}")
                    nc.sync.dma_start(out=Bb[:, :], in_=bcast_ap(s))
                    Cb = pC.tile([128, L], BF, tag="Cb", name=f"Cb{dk}{s}")
                    nc.sync.dma_start(out=Cb[:, :], in_=bcast_ap(48 + s))
                    for h in range(2):
                        g = h * 48 + s
                        dA = pAM.tile([128, L], BF, tag="AM",
                                      name=f"dA{dk}{s}{h}")
                        nc.scalar.activation(dA[:, :], t_dt[(dk, h)][:, :],
                                             AF.Exp, scale=A_sb[dk][:, g:g + 1])
                        Xt = pX.tile([128, L], BF, tag="X", name=f"X{dk}{s}{h}")
                        eng = nc.vector if (s * 2 + h) % 7 == 0 else nc.gpsimd
                        eng.tensor_tensor(Xt[:, :], t_dtu[(dk, h)][:, :],
                                          Bb[:, :], op=OP.mult)
                        Ht = pH.tile([128, L], BF, tag="H", name=f"H{dk}{s}{h}")
                        if dk == "f":
                            nc.vector.tensor_tensor_scan(
                                Ht[:, :], dA[:, :], Xt[:, :], 0.0,
                                OP.mult, OP.add)
                        else:
                            nc.vector.tensor_tensor_scan(
                                rev(Ht), rev(dA), rev(Xt), 0.0,
                                OP.mult, OP.add)
                        Mt = pAM.tile([128, L], BF, tag="AM",
                                      name=f"M{dk}{s}{h}")
                        nc.vector.tensor_tensor(Mt[:, :], Ht[:, :], Cb[:, :],
                                                op=OP.mult)
                        for c in range(NCH):
                            nc.tensor.matmul(p_y[h][:, C512[c]], ident_sb[:, :],
                                             Mt[:, C512[c]],
                                             start=(s == 0), stop=False)
                # fold u*Dp in as the final accumulated matmul (lhsT=diag(Dp))
                for h in range(2):
                    for c in range(NCH):
                        nc.tensor.matmul(p_y[h][:, C512[c]],
                                         dpd[dk][:, h * 128:(h + 1) * 128],
                                         t_u[(dk, h)][:, C512[c]],
                                         start=False, stop=True)
                if dk == "f":
                    for h in range(2):
                        yf = pers.tile([128, L], F32, tag=f"yf{h}",
                                       name=f"yf{h}")
                        nc.vector.tensor_copy(yf[:, :], p_y[h][:, :])
                        t_yf[h] = yf
                else:
                    # combine: yg = (yf + yb) * sz   (bf16)
                    for h in range(2):
                        yfb = pX.tile([128, L], BF, tag="X", name=f"yfb{h}")
                        nc.vector.tensor_tensor(yfb[:, :], t_yf[h][:, :],
                                                p_y[h][:, :], op=OP.add)
                        yg = pH.tile([128, L], BF, tag="H", name=f"yg{h}")
                        nc.vector.tensor_tensor(yg[:, :], yfb[:, :],
                                                t_sz[h][:, :], op=OP.mult)
                        t_yg[h] = yg

            # ---------------- out proj + residual -------------------------
            t_out = bigf.tile([128, L], F32, tag="bigf", name="outsb")
            if _phases != "all":
                t_yg = {h: t_u[("f", h)] for h in range(2)}  # timing probe only
            po = ps.tile([128, L], F32, tag="ps", name="po")
            for c in range(NCH):
                for h in range(2):
                    nc.tensor.matmul(po[:, C512[c]], wout_sb[:, h * 128:(h + 1) * 128],
                                     t_yg[h][:, C512[c]],
                                     start=(h == 0), stop=(h == 1))
                nc.vector.tensor_tensor(t_out[:, C512[c]], po[:, C512[c]],
                                        t_x[:, C512[c]], op=OP.add)
            nc.sync.dma_start(out=d_out[:, :], in_=t_out[:, :])

    nc.compile()
    return nc


# ---------------------------------------------------------------- host -----
def _prep_weights(inputs):
    import ml_dtypes
    f = np.float32
    bf = ml_dtypes.bfloat16
    ip = inputs["in_proj_w"].astype(f)          # (512, 128)
    xc_w = ip[:D_INNER]                          # (256, 128)
    z_w = ip[D_INNER:]                           # (256, 128)

    def conv_fold(conv_w):
        # wci[p, (k*2+h)*128 + m] = conv_w[h*128+m, 0, k] * xc_w[h*128+m, p]
        out = np.zeros((128, 1024), f)
        for k in range(4):
            for h in range(2):
                blk = (conv_w[h * 128:(h + 1) * 128, 0, k][:, None]
                       * xc_w[h * 128:(h + 1) * 128, :])   # (128m, 128p)
                out[:, (k * 2 + h) * 128:(k * 2 + h + 1) * 128] = blk.T
        return out

    def xproj_pack(w):                           # w: (104, 256)
        out = np.zeros((128, 208), f)
        for h in range(2):
            out[:, h * 104:(h + 1) * 104] = w[:, h * 128:(h + 1) * 128].T
        return out.astype(bf)

    wz = np.zeros((128, 256), f)
    for h in range(2):
        wz[:, h * 128:(h + 1) * 128] = z_w[h * 128:(h + 1) * 128, :].T
    wout = np.zeros((128, 256), f)
    op = inputs["out_proj_w"].astype(f)          # (128, 256)
    for h in range(2):
        wout[:, h * 128:(h + 1) * 128] = op[:, h * 128:(h + 1) * 128].T

    def A_pack(A_log):
        A = -np.exp(A_log.astype(f))             # (256, 48)
        out = np.zeros((128, 96), f)
        for h in range(2):
            out[:, h * 48:(h + 1) * 48] = A[h * 128:(h + 1) * 128, :]
        return out

    def halves(v):                               # (256,) -> (128, 2)
        return np.stack([v[:128], v[128:]], axis=1).astype(f)

    def dp_diag(v):                              # (256,) -> [128, 256] bf16
        out = np.zeros((128, 256), f)
        out[:, 0:128] = np.diag(v[:128])
        out[:, 128:256] = np.diag(v[128:])
        return out.astype(bf)

    cols = np.zeros((128, 12), f)
    cols[:, 0] = inputs["ln_w"]
    cols[:, 1] = inputs["ln_b"]
    cols[:, 2:4] = halves(inputs["dt_proj_b"])
    cols[:, 4:6] = halves(inputs["dt_proj_b_b"])
    cols[:, 6:8] = halves(inputs["conv_b"])
    cols[:, 8:10] = halves(inputs["conv_b_b"])
    cols[:, 10] = EPS
    cols[:, 11] = 1.0

    w = {
        "wci_f": conv_fold(inputs["conv_w"].astype(f)),
        "wci_b": conv_fold(inputs["conv_w_b"].astype(f)),
        "wz": wz,
        "wx_f": xproj_pack(inputs["x_proj_w"].astype(f)),
        "wx_b": xproj_pack(inputs["x_proj_w_b"].astype(f)),
        "wdt_f": inputs["dt_proj_w"].astype(f).T.copy().astype(bf),  # (8,256)
        "wdt_b": inputs["dt_proj_w_b"].astype(f).T.copy().astype(bf),
        "wout": wout.astype(bf),
        "A_f": A_pack(inputs["A_log"]),
        "A_b": A_pack(inputs["A_b_log"]),
        "cols": cols,
        "ones128": np.full((128, 128), 1.0 / 128.0, f),
        "ones_1x128": np.ones((1, 128), f),
        "identv": np.eye(128).astype(bf),
        "dpd_f": dp_diag(inputs["Dp"].astype(f)),
        "dpd_b": dp_diag(inputs["Dp_b"].astype(f)),
    }
    return w


def _get_nc():
    if "nc" not in _CACHE:
        _CACHE["nc"] = _build_nc()
    return _CACHE["nc"]


def _make_runner():
    """Build a cached jitted 8-core runner (mirrors run_bass_via_pjrt)."""
    import jax
    from jax.sharding import Mesh, PartitionSpec
    from jax.experimental.shard_map import shard_map
    from concourse import bass2jax, mybir

    nc = _get_nc()
    bass2jax.install_neuronx_cc_hook()
    part_name = nc.partition_id_tensor.name if nc.partition_id_tensor else None
    in_names, out_names, out_avals, zero_shapes = [], [], [], []
    for alloc in nc.m.functions[0].allocations:
        if not isinstance(alloc, mybir.MemoryLocationSet):
            continue
        name = alloc.memorylocations[0].name
        if alloc.kind == "ExternalInput":
            if name != part_name:
                in_names.append(name)
        elif alloc.kind == "ExternalOutput":
            out_names.append(name)
            shape = tuple(alloc.tensor_shape)
            dtype = mybir.dt.np(alloc.dtype)
            out_avals.append(jax.core.ShapedArray(shape, dtype))
            zero_shapes.append((shape, dtype))
    n_params = len(in_names)
    all_names = in_names + out_names
    if part_name is not None:
        all_names = all_names + [part_name]
    donate = tuple(range(n_params, n_params + len(out_names)))

    def _body(*args):
        operands = list(args)
        if part_name is not None:
            operands.append(bass2jax.partition_id_tensor())
        outs = bass2jax._bass_exec_p.bind(
            *operands,
            out_avals=tuple(out_avals),
            in_names=tuple(all_names),
            out_names=tuple(out_names),
            lowering_input_output_aliases=(),
            sim_require_finite=True,
            sim_require_nnan=True,
            nc=nc,
        )
        return tuple(outs)

    devices = jax.devices()[:N_CORES]
    mesh = Mesh(np.asarray(devices), ("core",))
    n_args = n_params + len(out_names)
    sharded = jax.jit(
        shard_map(_body, mesh=mesh,
                  in_specs=(PartitionSpec("core"),) * n_args,
                  out_specs=(PartitionSpec("core"),) * len(out_names),
                  check_rep=False),
        donate_argnums=donate, keep_unused=True)

    def run(in_maps):
        concat_in = [
            np.concatenate([np.asarray(in_maps[c][nm]) for c in range(N_CORES)],
                           axis=0)
            for nm in in_names
        ]
        concat_zeros = [np.zeros((N_CORES * sh[0], *sh[1:]), dt)
                        for sh, dt in zero_shapes]
        out_arrs = sharded(*concat_in, *concat_zeros)
        out_arrs = [np.asarray(a) for a in out_arrs]
        return [
            {nm: out_arrs[i].reshape(N_CORES, *out_avals[i].shape)[c]
             for i, nm in enumerate(out_names)}
            for c in range(N_CORES)
        ]

    return run


def _run(in_maps, n_iter=1):
    """Execute on 8 cores; returns (results, best_wall_ns)."""
    if "runner" not in _CACHE:
        _CACHE["runner"] = _make_runner()
    run = _CACHE["runner"]
    best = None
    results = None
    for _ in range(n_iter):
        t0 = time.perf_counter_ns()
        results = run(in_maps)
        dt_ns = time.perf_counter_ns() - t0
        best = dt_ns if best is None else min(best, dt_ns)
    return results, best


def kernel(**inputs):
    inputs = {k: np.asarray(v) for k, v in inputs.items()}
    w = _prep_weights(inputs)
    x = inputs["x"].astype(np.float32)           # (8, 2048, 128)
    in_maps = []
    for b in range(N_CORES):
        m = dict(w)
        m["x_T"] = np.ascontiguousarray(x[b].T)  # (128, 2048)
        in_maps.append(m)
    results, wall = _run(in_maps, n_iter=1)
    _CACHE["last_wall_ns"] = wall
    out = np.stack([results[b]["out_T"].T for b in range(N_CORES)])
    return out.astype(np.float32)
